# revision 52
# baseline (speedup 1.0000x reference)
"""Trainium2 Bass kernel for nn_CausalSelfAttention_90168543412719.

Sharding: head-parallel over the 32 attention heads (4 heads/core on 8
NeuronCores). Each core computes q/k/v projections for its heads from the
full x, runs causal + adapter-prefix + whisper cross attention for its
heads, then an AllToAll reshards y from head-sharded to token-sharded and
each core applies c_proj to its own 256 token rows. Whisper K/V MLP is
row-sharded across the 5 whisper cores.

All matmuls run in bf16 with fp32 PSUM accumulation. Host pre-slices /
pre-transposes / pre-casts every operand into the exact layout the PE
wants, so the device never transposes anything.

Phase order puts the qkv GEMM (pure PE work) first so the whisper-MLP
weight streams prefetch underneath it. Whisper keys are padded 1500->1536
so every kv loop runs 12 uniform 128-key tiles; the 36 tail keys are
killed with a per-partition bias of -30000 on the exp of the last tile.

Rope layout trick: the q/k head dims are permuted to [evens..., odds...]
(host permutes the corresponding weight columns), so rope becomes four
contiguous 64-partition block ops. Scores contract over the permuted dim
on both sides, so the permutation cancels; v / y stay in natural order.

Attention works in transposed score space: s_T[keys, q] = k_T.T @ q_T, so
probabilities come out in the exact [keys, q] layout the AV matmul wants
as rhs (no P transposes). Softmax denominators are column sums computed
on the PE with a ones vector; no max-shift is needed at these scales
(exp stays comfortably inside f32 range).
"""

import os
import sys
from contextlib import ExitStack

import numpy as np
import ml_dtypes

for _p in ("/root/.axon_site/_ro/trn_rl_repo", "/opt/trn_rl_repo"):
    if os.path.isdir(_p) and _p not in sys.path:
        sys.path.append(_p)

import concourse.bass as bass
import concourse.mybir as mybir
import concourse.tile as tile
from concourse.bass_utils import run_bass_kernel_spmd

BF16 = mybir.dt.bfloat16
F32 = mybir.dt.float32
NBF = ml_dtypes.bfloat16
AF = mybir.ActivationFunctionType
ALU = mybir.AluOpType

B, T, C = 2, 1024, 4096
NH, HS = 32, 128
NCORES, HPC = 8, 4  # heads per core
A_LEN = 10
AT, AD, DD = 1500, 1280, 80  # audio_t, audio_d, down dim
AT2 = 1536  # whisper keys padded to 12*128
NWH, WHD = 20, 64  # whisper heads / head dim
EPS = 1e-5
BT = B * T  # 2048 global tokens, b-major
TT = 512  # token tile (matmul free dim)
NTT = BT // TT  # 4
TPC = BT // NCORES  # 256 tokens per core for c_proj
SCALE = 1.0 / float(np.sqrt(HS))
NEG = -30000.0  # additive mask value pre-scale; exp(NEG*SCALE) == 0 in f32
NKT = AT2 // 128  # 12 whisper key tiles per batch
KO = C // 128  # 32 contraction tiles over C
NOT = AD // 128  # 10 whisper tiles over AD

PERM = np.concatenate([np.arange(0, HS, 2), np.arange(1, HS, 2)])  # 128
PERM64 = np.concatenate([np.arange(0, WHD, 2), np.arange(1, WHD, 2)])  # 64

_PROG_CACHE = {}
_MAX_WAITS = 1


def _split_multi_waits(nc):
    """walrus here rejects >1 semaphore wait per instruction; hoist extras
    onto preceding NoOps on the same engine."""
    for f in nc.m.functions:
        for blk in f.blocks:
            insts = list(blk.instructions)
            new = []
            changed = False
            for inst in insts:
                si = inst.sync_info
                if si is not None and si.on_wait and len(si.on_wait) > _MAX_WAITS:
                    waits = list(si.on_wait)
                    keep = waits[-_MAX_WAITS:]
                    extra = waits[:-_MAX_WAITS]
                    for i in range(0, len(extra), _MAX_WAITS):
                        new.append(
                            mybir.InstNoOp(
                                name=f"{inst.name}.wsplit{i}",
                                engine=inst.engine,
                                debug=inst.debug,
                                sync_info=mybir.SyncInfo(
                                    on_wait=extra[i : i + _MAX_WAITS], on_update=[]
                                ),
                                bass_nofuse=True,
                            )
                        )
                    inst.sync_info = mybir.SyncInfo(
                        on_wait=keep, on_update=list(si.on_update)
                    )
                    changed = True
                new.append(inst)
            if changed:
                try:
                    blk.instructions[:] = new
                except TypeError:
                    blk.instructions = new


def build_program(gating_factor: float, proj_gating: float) -> bass.Bass:
    nc = bass.Bass()

    # ---------------- I/O (per-core data arrives via in_maps)
    xT = nc.dram_tensor("xT", [C, BT], BF16, kind="ExternalInput")
    wq = nc.dram_tensor("wq", [C, HPC * HS], BF16, kind="ExternalInput")
    wk = nc.dram_tensor("wk", [C, HPC * HS], BF16, kind="ExternalInput")
    wv = nc.dram_tensor("wv", [C, HPC * HS], BF16, kind="ExternalInput")
    cosT = nc.dram_tensor("cosT", [HS // 2, T], F32, kind="ExternalInput")
    sinT = nc.dram_tensor("sinT", [HS // 2, T], F32, kind="ExternalInput")
    masks = nc.dram_tensor("masks", [128, 128], F32, kind="ExternalInput")
    akT = nc.dram_tensor("akT", [HPC, HS, A_LEN], BF16, kind="ExternalInput")
    avd = nc.dram_tensor("avd", [HPC, A_LEN, HS], BF16, kind="ExternalInput")
    aTd = nc.dram_tensor("aT", [AD, B * 300], BF16, kind="ExternalInput")
    wkey = nc.dram_tensor("wkey", [AD, AD], BF16, kind="ExternalInput")
    wval = nc.dram_tensor("wval", [AD, AD], BF16, kind="ExternalInput")
    vbias = nc.dram_tensor("vbias", [128, NOT], F32, kind="ExternalInput")
    rmsk = nc.dram_tensor("rmsk", [128, NOT], F32, kind="ExternalInput")
    rmsv = nc.dram_tensor("rmsv", [128, NOT], F32, kind="ExternalInput")
    pdown = nc.dram_tensor("pdown", [AD, DD], BF16, kind="ExternalInput")
    pupk = nc.dram_tensor("pupk", [DD, 20 * WHD], BF16, kind="ExternalInput")
    pupv = nc.dram_tensor("pupv", [DD, AD], BF16, kind="ExternalInput")
    padkT = nc.dram_tensor("padkT", [B, HS, AT2], BF16, kind="ExternalInput")
    padvT = nc.dram_tensor("padvT", [B, 128, NKT, WHD], BF16, kind="ExternalInput")
    padv0 = nc.dram_tensor("padv0", [B, 128, NKT, WHD], BF16, kind="ExternalInput")
    cproj = nc.dram_tensor("cproj", [C, C], BF16, kind="ExternalInput")
    out = nc.dram_tensor("out", [TPC, C], F32, kind="ExternalOutput")

    gf = float(gating_factor)
    pg = float(proj_gating)

    with tile.TileContext(nc) as tc, ExitStack() as ctx:
        dram = ctx.enter_context(tc.tile_pool(name="dram", bufs=1, space="DRAM"))
        const = ctx.enter_context(tc.tile_pool(name="const", bufs=1))
        persist = ctx.enter_context(tc.tile_pool(name="persist", bufs=1))

        # Collective bounce (split per batch) + whisper pv staging in DRAM
        a2a0_in = dram.tile([NCORES, HPC * HS, 128], BF16)
        a2a0_out = dram.tile([NCORES, HPC * HS, 128], BF16)
        a2a1_in = dram.tile([NCORES, HPC * HS, 128], BF16)
        a2a1_out = dram.tile([NCORES, HPC * HS, 128], BF16)
        a2a_ins = [a2a0_in, a2a1_in]
        a2a_outs = [a2a0_out, a2a1_out]
        pv_d = dram.tile([B, HPC, AT2 * WHD], BF16)  # per-(b,head) flat pv rows

        ones_bf = const.tile([128, 1], BF16)
        nc.gpsimd.memset(ones_bf[:], 1.0)
        ones_row = const.tile([1, 128], BF16)
        nc.gpsimd.memset(ones_row[:], 1.0)
        ones128 = const.tile([128, 128], BF16)
        nc.gpsimd.memset(ones128[:], 1.0)
        eps_sb = const.tile([1, 1], F32)
        nc.gpsimd.memset(eps_sb[:], EPS)
        tailb = const.tile([128, 1], F32)  # kill keys 1500:1536 in last tile
        nc.gpsimd.memset(tailb[:], NEG)
        nc.gpsimd.memset(tailb[0 : AT - 11 * 128, :], 0.0)
        zrow = const.tile([AT2 - AT, WHD], BF16)
        nc.gpsimd.memset(zrow[:], 0.0)

        # SBUF state persisting through attention (freed before phase P)
        mid = ctx.enter_context(ExitStack())
        midp = mid.enter_context(tc.tile_pool(name="midp", bufs=1))
        qT_sb = midp.tile([128, HPC, NTT, TT], BF16)  # roped q, permuted dims
        kT_sb = midp.tile([128, HPC, NTT, TT], BF16)  # roped k, permuted dims
        v_sb = midp.tile([128, NTT, 4, HPC * HS], BF16)  # [tok128, tt, st, cols]
        cos_sb = const.tile([64, T], F32)
        sin_sb = const.tile([64, T], F32)
        mask_sb = const.tile([128, 128], F32)
        akT_sb = const.tile([128, HPC, A_LEN], BF16)
        av_sb = const.tile([A_LEN, HPC, HS], BF16)
        dk_loc = persist.tile([DD, B * 300], BF16)  # whisper down-proj, own rows
        dv_loc = persist.tile([DD, B * 300], BF16)

        # W1 constants (outside Q's pools so the DMAs overlap Q).
        # mid closes whc/ostp/midp before phase P to free their SBUF.
        whc = mid.enter_context(tc.tile_pool(name="whc", bufs=1))
        aT_sb = whc.tile([128, NOT, B * 300], BF16)
        pdown_sb = whc.tile([128, NOT, DD], BF16)
        vb_sb = whc.tile([128, NOT], F32)
        rmsk_sb = whc.tile([128, NOT], F32)
        rmsv_sb = whc.tile([128, NOT], F32)
        pupv_sb = whc.tile([DD, AD], BF16)

        def deferred_const_dmas():
            # issued after the first Q tiles so the critical first matmul
            # chain is not starved by prefetch traffic
            nc.sync.dma_start(cos_sb[:], cosT[:])
            nc.sync.dma_start(sin_sb[:], sinT[:])
            nc.sync.dma_start(mask_sb[:], masks[:])
            nc.sync.dma_start(akT_sb[:], akT[:].rearrange("h p a -> p h a"))
            nc.sync.dma_start(av_sb[:], avd[:].rearrange("h a d -> a h d"))
            nc.sync.dma_start(aT_sb[:], aTd[:].rearrange("(ko p) r -> p ko r", p=128))
            nc.sync.dma_start(pdown_sb[:], pdown[:].rearrange("(ko p) n -> p ko n", p=128))
            nc.sync.dma_start(vb_sb[:], vbias[:])
            nc.sync.dma_start(rmsk_sb[:], rmsk[:])
            nc.sync.dma_start(rmsv_sb[:], rmsv[:])
            nc.sync.dma_start(pupv_sb[:], pupv[:])

        # =============== Phase Q: qkv projection + rope
        with (
            tc.tile_pool(name="qx", bufs=2) as qx,
            tc.tile_pool(name="qw", bufs=2) as qw,
            tc.tile_pool(name="qwv", bufs=1) as qwv,
            tc.tile_pool(name="qp", bufs=3, space="PSUM") as qp,
            tc.tile_pool(name="qt", bufs=2) as qtp,
        ):
            wv_w = qwv.tile([128, KO, HPC * HS], BF16)
            for tt in range(NTT):
                x_t = qx.tile([128, KO, TT], BF16, tag="x_t")
                nc.sync.dma_start(
                    x_t[:],
                    xT[:, tt * TT : (tt + 1) * TT].rearrange("(ko p) t -> p ko t", p=128),
                )
                co = (tt % 2) * TT  # rope position offset within batch
                for ph in range(2):  # 0: q, 1: k
                    wsrc = wq if ph == 0 else wk
                    dst = qT_sb if ph == 0 else kT_sb
                    for hl in range(HPC):
                        w_t = qw.tile([128, KO, HS], BF16, tag="w_t")
                        nc.sync.dma_start(
                            w_t[:],
                            wsrc[:, hl * HS : (hl + 1) * HS].rearrange(
                                "(ko p) n -> p ko n", p=128
                            ),
                        )
                        if tt == 0 and ph == 0 and hl == 0:
                            nc.sync.dma_start(
                                wv_w[:], wv[:].rearrange("(ko p) n -> p ko n", p=128)
                            )
                            deferred_const_dmas()
                        ps = qp.tile([128, TT], F32, tag="qk_ps")
                        for ko in range(KO):
                            nc.tensor.matmul(
                                ps[:], w_t[:, ko, :], x_t[:, ko, :],
                                start=(ko == 0), stop=(ko == KO - 1),
                            )
                        # rope on [evens|odds] halves
                        ev, od = ps[0:64, :], ps[64:128, :]
                        cs = cos_sb[:, co : co + TT]
                        sn = sin_sb[:, co : co + TT]
                        t1 = qtp.tile([64, TT], F32, tag="r1")
                        t2 = qtp.tile([64, TT], F32, tag="r2")
                        nc.vector.tensor_tensor(t1[:], ev, cs, ALU.mult)
                        nc.vector.tensor_tensor(t2[:], od, sn, ALU.mult)
                        nc.vector.tensor_sub(dst[0:64, hl, tt, :], t1[:], t2[:])
                        nc.vector.tensor_tensor(t1[:], od, cs, ALU.mult)
                        nc.vector.tensor_tensor(t2[:], ev, sn, ALU.mult)
                        nc.vector.tensor_add(dst[64:128, hl, tt, :], t1[:], t2[:])
                for st in range(4):  # v: [tok128, cols512]
                    ps = qp.tile([128, HPC * HS], F32, tag="v_ps")
                    for ko in range(KO):
                        nc.tensor.matmul(
                            ps[:],
                            x_t[:, ko, st * 128 : (st + 1) * 128],
                            wv_w[:, ko, :],
                            start=(ko == 0), stop=(ko == KO - 1),
                        )
                    nc.scalar.copy(v_sb[:, tt, st, :], ps[:])

        # causal+adapter partial y, held until the whisper branch adds in.
        # Allocated after Q's pools close so it reuses their SBUF space.
        ostp = mid.enter_context(tc.tile_pool(name="ostp", bufs=1))
        o_store = ostp.tile([128, B * HPC * 2, TT], BF16)

        # =============== Phases A1+A2: attention. A1 does causal+adapter
        # into o_store, with whisper-MLP (W1), pv up-projection (W2) and
        # the b=0 pk assembly interleaved as filler work. A2 does whisper
        # cross-attention, with b=1 prep interleaved into b=0's slots, and
        # launches each batch's AllToAll as soon as it is staged.
        with (
            tc.tile_pool(name="w2", bufs=2) as w2,
            tc.tile_pool(name="ap", bufs=2) as ap,
            tc.tile_pool(name="apk", bufs=2) as apk,
            tc.tile_pool(name="apv", bufs=2) as apv,
            tc.tile_pool(name="ascp", bufs=2, space="PSUM") as ascp,
            tc.tile_pool(name="ayp", bufs=2, space="PSUM") as ayp,
            tc.tile_pool(name="adp", bufs=1, space="PSUM") as adp,
        ):
            # W1-only pools live in a nested scope freed after the A1
            # fillers drain, returning 3 PSUM banks + ~25KB/part of SBUF
            # for the A2 rep pool and the early cproj stream.
            w1s = ExitStack()
            wh = w1s.enter_context(tc.tile_pool(name="wh", bufs=1))
            whs = w1s.enter_context(tc.tile_pool(name="whs", bufs=2))
            whp_h = w1s.enter_context(tc.tile_pool(name="whp_h", bufs=1, space="PSUM"))
            whp_s = w1s.enter_context(tc.tile_pool(name="whp_s", bufs=1, space="PSUM"))
            whp_m = w1s.enter_context(tc.tile_pool(name="whp_m", bufs=1, space="PSUM"))
            pupk_sb = apk.tile([DD, 20, WHD], BF16, tag="pupk")
            nc.sync.dma_start(pupk_sb[:], pupk[:].rearrange("d (u i) -> d u i", i=WHD))

            w1_state = {}
            pk4_t = {}

            def prefetch_w(kv, ot):
                w_t = whs.tile([128, NOT, 128], BF16, tag="wh_w")
                w_dram = wkey if kv == 0 else wval
                nc.sync.dma_start(
                    w_t[:],
                    w_dram[:, ot * 128 : (ot + 1) * 128].rearrange(
                        "(ko p) n -> p ko n", p=128
                    ),
                )
                w1_state["w_next"] = w_t

            def h_unit(kv, ot):
                w_t = w1_state["w_next"]
                if ot + 1 < NOT:
                    prefetch_w(kv, ot + 1)
                elif kv == 0:
                    prefetch_w(1, 0)
                if ot == 0:
                    h_cur = wh.tile([128, NOT, B * 300], BF16, tag="h_sb")
                    ssq_cur = whp_s.tile([33, 300], F32, tag="ssq")
                    w1_state["h"] = h_cur
                    w1_state["ssq"] = ssq_cur
                h_sb = w1_state["h"]
                ssq = w1_state["ssq"]
                for b2 in range(2):
                    c0 = 300 * b2
                    hp = whp_h.tile([128, 300], F32, tag="hps")
                    for kt in range(NOT):
                        nc.tensor.matmul(
                            hp[:],
                            w_t[:, kt, :],
                            aT_sb[:, kt, c0 : c0 + 300],
                            start=(kt == 0), stop=(kt == NOT - 1),
                        )
                    if kv == 1:
                        nc.scalar.activation(
                            h_sb[:, ot, c0 : c0 + 300], hp[:], AF.Identity,
                            bias=vb_sb[:, ot : ot + 1],
                        )
                    else:
                        nc.scalar.copy(h_sb[:, ot, c0 : c0 + 300], hp[:])
                    hsq = wh.tile([128, 300], BF16, tag="hsq")
                    nc.scalar.activation(hsq[:], h_sb[:, ot, c0 : c0 + 300], AF.Square)
                    sr = 32 * b2
                    nc.tensor.matmul(
                        ssq[sr : sr + 1, :], ones_bf[:], hsq[:],
                        start=(ot == 0), stop=(ot == NOT - 1),
                        skip_group_check=True,
                    )

            def w1_tail(kv):
                rms_w = rmsk_sb if kv == 0 else rmsv_sb
                d_dst = dk_loc if kv == 0 else dv_loc
                h_sb = w1_state["h"]
                ssq = w1_state["ssq"]
                for b2 in range(2):
                    c0 = 300 * b2
                    sq_sb = wh.tile([1, 300], F32, tag="sq_sb")
                    sr = 32 * b2
                    nc.scalar.activation(
                        sq_sb[:], ssq[sr : sr + 1, :], AF.Sqrt,
                        bias=eps_sb[:], scale=1.0 / AD,
                    )
                    rr_sb = wh.tile([1, 300], F32, tag="rr_sb")
                    nc.vector.reciprocal(rr_sb[:], sq_sb[:])
                    rr_bf = wh.tile([1, 300], BF16, tag="rr_bf")
                    nc.vector.tensor_copy(rr_bf[:], rr_sb[:])
                    m1 = whp_m.tile([128, TT], F32, tag="m")
                    nc.tensor.matmul(m1[:, 0:300], ones_row[:], rr_bf[:], start=True, stop=True)
                    rrb = wh.tile([128, 300], F32, tag="rrb")
                    nc.vector.tensor_copy(rrb[:], m1[:, 0:300])
                    hn_sb = wh.tile([128, NOT, 300], BF16, tag="hn_sb")
                    for ot in range(NOT):
                        nc.vector.scalar_tensor_tensor(
                            hn_sb[:, ot, :], h_sb[:, ot, c0 : c0 + 300],
                            rms_w[:, ot : ot + 1], rrb[:], ALU.mult, ALU.mult,
                        )
                    m2 = whp_m.tile([128, TT], F32, tag="m")
                    for kt in range(NOT):
                        nc.tensor.matmul(
                            m2[0:DD, 0:300], pdown_sb[:, kt, :], hn_sb[:, kt, :],
                            start=(kt == 0), stop=(kt == NOT - 1),
                        )
                    nc.scalar.activation(d_dst[:, c0 : c0 + 300], m2[0:DD, 0:300], AF.Silu)

            def w2_unit(b, hl):
                # pv rows: head g keys [1500, 64] are wv rows [75g, 75g+75)
                # flat row-major; rows 1500:1536 (tail) zeroed.
                wvrow = w2.tile([128, AD], BF16, tag="wvrow")
                for ns in range(3):
                    n0 = ns * 512
                    nsz = min(512, AD - n0)
                    ps = ascp.tile([128, TT], F32, tag="sc")
                    nc.tensor.matmul(
                        ps[0:75, :nsz],
                        dv_loc[:, b * 300 + 75 * hl : b * 300 + 75 * (hl + 1)],
                        pupv_sb[:, n0 : n0 + nsz],
                        start=True, stop=True,
                    )
                    nc.scalar.copy(wvrow[0:75, n0 : n0 + nsz], ps[0:75, :nsz])
                nc.sync.dma_start(
                    pv_d[b, hl, : AT * WHD].rearrange("(r d) -> r d", r=75),
                    wvrow[0:75, :],
                )
                nc.sync.dma_start(
                    pv_d[b, hl, AT * WHD :].rearrange("(r d) -> r d", r=AT2 - AT),
                    zrow[:],
                )

            def pk4_start(b):
                # pk for all 4 heads: [128 dims, hl, 1536 keys].
                # pk[i, hl, 20*jr+u] = pad[i, key] (whisper rows host-zeroed)
                #                    + pupk[:, u, i] . dk[:, 75*hl+jr]
                pk4 = apk.tile([128, HPC, AT2], BF16, tag="pk4")
                pk4_t[b] = pk4
                for hl in range(HPC):
                    nc.sync.dma_start(pk4[:, hl, :], padkT[b, :, :])

            def pk4_chunk(b, u0):
                # adds run on the idle gpsimd engine so they neither load
                # the DVE nor head-of-line-block the PE stream
                pk4 = pk4_t[b]
                pk4v = pk4[:, :, :AT].rearrange("p h (j u) -> p h j u", u=20)
                dk4 = dk_loc[:, b * 300 : (b + 1) * 300]
                for u in range(u0, u0 + 5):
                    pkp = ascp.tile([128, TT], F32, tag="sc")
                    nc.tensor.matmul(
                        pkp[0:32, 0:300], pupk_sb[:, u, 0:32], dk4,
                        start=True, stop=True,
                    )
                    nc.tensor.matmul(
                        pkp[64:96, 0:300], pupk_sb[:, u, 32:64], dk4,
                        start=True, stop=True,
                    )
                    for half in range(2):
                        p0 = 64 * half
                        nc.vector.tensor_add(
                            pk4v[p0 : p0 + 32, :, :, u],
                            pkp[p0 : p0 + 32, 0:300].rearrange(
                                "p (h j) -> p h j", h=HPC
                            ),
                            pk4v[p0 : p0 + 32, :, :, u],
                        )

            prefetch_w(0, 0)
            fillers = []
            for ot in range(NOT):
                fillers.append(lambda ot=ot: h_unit(0, ot))
            fillers.append(lambda: w1_tail(0))
            # pk assembly right after dk is ready so the gpsimd add chain
            # drains long before A2's first scores read pk4
            fillers.append(lambda: pk4_start(0))
            for u0 in range(0, 20, 5):
                fillers.append(lambda u0=u0: pk4_chunk(0, u0))
            for ot in range(NOT):
                fillers.append(lambda ot=ot: h_unit(1, ot))
            fillers.append(lambda: w1_tail(1))
            for hl in range(HPC):
                fillers.append(lambda hl=hl: w2_unit(0, hl))

            n_slots = B * HPC * 2
            fidx = 0

            # ---------------- A1: causal + adapter. The causal
            # normalization tail is deferred until after both qt kt-loops
            # so the PE stream never waits on the DVE recip chain; the
            # adapter tail (ready almost immediately) stays inline.
            for b in range(B):
                for hl in range(HPC):
                    dt = adp.tile([65, TT], F32, tag="den")
                    ys = {}
                    tmps = {}
                    for qt in range(2):
                        qcol = qT_sb[:, hl, 2 * b + qt, :]  # [128, 512]
                        nkt = 4 * (qt + 1)
                        y_ps = ayp.tile([128, TT], F32, tag="y")
                        dr = 64 * qt
                        pts = {}
                        for kt in range(nkt):
                            sp = ascp.tile([128, TT], F32, tag="sc")
                            nc.tensor.matmul(
                                sp[:],
                                kT_sb[:, hl, 2 * b + kt // 4,
                                      (kt % 4) * 128 : (kt % 4) * 128 + 128],
                                qcol, start=True, stop=True,
                            )
                            pt = ap.tile([128, TT], BF16, tag="pt")
                            roff = kt * 128 - qt * TT
                            if roff >= 0:  # diagonal block
                                # columns < roff are fully masked: zero them
                                # in pt (off the critical path); add the
                                # shared [128,128] triangular mask on the
                                # boundary column block; exp the rest.
                                if roff > 0:
                                    nc.gpsimd.memset(pt[:, 0:roff], 0.0)
                                nc.vector.tensor_add(
                                    sp[:, roff : roff + 128],
                                    sp[:, roff : roff + 128],
                                    mask_sb[:],
                                )
                                nc.scalar.activation(
                                    pt[:, roff:TT], sp[:, roff:TT], AF.Exp, scale=SCALE
                                )
                            else:
                                nc.scalar.activation(pt[:], sp[:], AF.Exp, scale=SCALE)
                            pts[kt] = pt
                            if kt >= 1:
                                ptp = pts.pop(kt - 1)
                                nc.tensor.matmul(
                                    dt[dr : dr + 1, :], ones_bf[:], ptp[:],
                                    start=(kt - 1 == 0), stop=False,
                                    skip_group_check=True,
                                )
                                nc.tensor.matmul(
                                    y_ps[:],
                                    v_sb[:, 2 * b + (kt - 1) // 4, (kt - 1) % 4,
                                         hl * HS : (hl + 1) * HS],
                                    ptp[:],
                                    start=(kt - 1 == 0), stop=False,
                                )
                        ptp = pts.pop(nkt - 1)
                        nc.tensor.matmul(
                            dt[dr : dr + 1, :], ones_bf[:], ptp[:],
                            start=False, stop=True, skip_group_check=True,
                        )
                        nc.tensor.matmul(
                            y_ps[:],
                            v_sb[:, 2 * b + (nkt - 1) // 4, (nkt - 1) % 4,
                                 hl * HS : (hl + 1) * HS],
                            ptp[:],
                            start=False, stop=True,
                        )
                        ys[qt] = y_ps
                        # ---- adapter prefix attention (tail inline: its
                        # denominator is ready right away; den row 32 is
                        # reused across qt with disjoint lifetimes)
                        sa = ascp.tile([128, TT], F32, tag="sc")
                        nc.tensor.matmul(
                            sa[0:A_LEN, :], akT_sb[:, hl, :], qcol, start=True, stop=True
                        )
                        pa = ap.tile([A_LEN, TT], BF16, tag="pa")
                        nc.scalar.activation(pa[:], sa[0:A_LEN, :], AF.Exp, scale=SCALE)
                        nc.tensor.matmul(
                            dt[32:33, :], ones_bf[0:A_LEN, :], pa[:],
                            start=True, stop=True, skip_group_check=True,
                        )
                        ya = ayp.tile([128, TT], F32, tag="y")
                        nc.tensor.matmul(ya[:], av_sb[:, hl, :], pa[:], start=True, stop=True)
                        ra = ap.tile([33, TT], F32, tag="rc")
                        nc.vector.reciprocal(ra[32:33, :], dt[32:33, :])
                        ra_bf = ap.tile([33, TT], BF16, tag="rcbf")
                        nc.vector.tensor_copy(ra_bf[32:33, :], ra[32:33, :])
                        rep2 = ascp.tile([128, TT], F32, tag="sc")
                        nc.tensor.matmul(
                            rep2[:], ones128[32:33, :], ra_bf[32:33, :],
                            start=True, stop=True,
                        )
                        rep_sb2 = ap.tile([128, TT], F32, tag="repsb")
                        nc.scalar.copy(rep_sb2[:], rep2[:])
                        tmp = ap.tile([128, TT], F32, tag="tmp")
                        nc.vector.tensor_tensor(tmp[:], ya[:], rep_sb2[:], ALU.mult)
                        tmps[qt] = tmp

                    rc = ap.tile([65, TT], F32, tag="rc")
                    rc_bf = ap.tile([65, TT], BF16, tag="rcbf")
                    for qt in range(2):
                        dr = 64 * qt
                        nc.vector.reciprocal(rc[dr : dr + 1, :], dt[dr : dr + 1, :])
                        nc.vector.tensor_copy(rc_bf[dr : dr + 1, :], rc[dr : dr + 1, :])
                    for qt in range(2):
                        oi = (b * HPC + hl) * 2 + qt
                        dr = 64 * qt
                        rep = ascp.tile([128, TT], F32, tag="sc")
                        nc.tensor.matmul(
                            rep[:], ones128[dr : dr + 1, :], rc_bf[dr : dr + 1, :],
                            start=True, stop=True,
                        )
                        rep_sb = ap.tile([128, TT], F32, tag="repsb")
                        nc.scalar.copy(rep_sb[:], rep[:])
                        o_sb = ap.tile([128, TT], F32, tag="o_sb")
                        nc.vector.tensor_tensor(o_sb[:], ys[qt][:], rep_sb[:], ALU.mult)
                        nc.vector.scalar_tensor_tensor(
                            o_store[:, oi, :], tmps[qt][:], gf, o_sb[:], ALU.mult, ALU.add
                        )
                        # interleave whisper-MLP / pv / pk-assembly fillers
                        slot = (b * HPC + hl) * 2 + qt
                        want = ((slot + 1) * len(fillers)) // n_slots
                        while fidx < want:
                            fillers[fidx]()
                            fidx += 1
            while fidx < len(fillers):
                fillers[fidx]()
                fidx += 1
            w1s.close()
            arps = ExitStack()
            arp = arps.enter_context(tc.tile_pool(name="arp", bufs=2, space="PSUM"))
            pt2 = arps.enter_context(tc.tile_pool(name="pt2", bufs=4))

            # ---------------- A2: whisper cross attention
            a2f = []
            a2f.append(lambda: pk4_start(1))
            for hl in range(HPC):
                a2f.append(lambda hl=hl: w2_unit(1, hl))
            for u0 in range(0, 20, 5):
                a2f.append(lambda u0=u0: pk4_chunk(1, u0))
            a2n = 0

            for b in range(B):
                pk4 = pk4_t[b]
                for hl in range(HPC):
                    # pv [keys, kt, 128d]: cols 0:64 whisper rows (+pad for
                    # non-whisper cores via padv0), cols 64:128 pad.
                    pv4 = apv.tile([128, NKT, HS], BF16, tag="pv")
                    nc.sync.dma_start(
                        pv4[:, :, 0:WHD],
                        pv_d[b, hl, :].rearrange("(kt p d) -> p kt d", p=128, d=WHD),
                    )
                    nc.sync.dma_start(pv4[:, :, WHD:HS], padvT[b, :, :, :])
                    p0t = apv.tile([128, NKT, WHD], BF16, tag="p0t")
                    nc.sync.dma_start(p0t[:], padv0[b, :, :, :])
                    nc.vector.tensor_add(pv4[:, :, 0:WHD], p0t[:], pv4[:, :, 0:WHD])

                    dt = adp.tile([65, TT], F32, tag="den")
                    # both query tiles' kt-loops interleaved: two independent
                    # score->exp->den/AV streams keep the PE fed while the
                    # Activation engine works through the exps
                    yw0 = ayp.tile([128, TT], F32, tag="y")
                    yw1 = ayp.tile([128, TT], F32, tag="y")
                    yws = {0: yw0, 1: yw1}
                    qcols = [qT_sb[:, hl, 2 * b + qt, :] for qt in range(2)]
                    pws = {}
                    for kt in range(NKT):
                        k0 = kt * 128
                        for qt in range(2):
                            sw = ascp.tile([128, TT], F32, tag="sc")
                            nc.tensor.matmul(
                                sw[:], pk4[:, hl, k0 : k0 + 128], qcols[qt],
                                start=True, stop=True,
                            )
                            pw = pt2.tile([128, TT], BF16, tag="pt2")
                            if kt == NKT - 1:
                                nc.scalar.activation(
                                    pw[:], sw[:], AF.Exp, bias=tailb[:], scale=SCALE
                                )
                            else:
                                nc.scalar.activation(pw[:], sw[:], AF.Exp, scale=SCALE)
                            pws[(qt, kt)] = pw
                        if kt >= 1:
                            for qt in range(2):
                                dr = 64 * qt
                                pwp_ = pws.pop((qt, kt - 1))
                                nc.tensor.matmul(
                                    dt[dr : dr + 1, :], ones_bf[:], pwp_[:],
                                    start=(kt - 1 == 0), stop=False,
                                    skip_group_check=True,
                                )
                                nc.tensor.matmul(
                                    yws[qt][:], pv4[:, kt - 1, :], pwp_[:],
                                    start=(kt - 1 == 0), stop=False,
                                )
                    for qt in range(2):
                        dr = 64 * qt
                        pwp_ = pws.pop((qt, NKT - 1))
                        nc.tensor.matmul(
                            dt[dr : dr + 1, :], ones_bf[:], pwp_[:],
                            start=False, stop=True, skip_group_check=True,
                        )
                        nc.tensor.matmul(
                            yws[qt][:], pv4[:, NKT - 1, :], pwp_[:], start=False, stop=True
                        )

                    rw = ap.tile([128, TT], F32, tag="rc")
                    rw_bf = ap.tile([128, TT], BF16, tag="rcbf")
                    for qt in range(2):
                        dr = 64 * qt
                        nc.vector.reciprocal(rw[dr : dr + 1, :], dt[dr : dr + 1, :])
                        nc.vector.tensor_copy(rw_bf[dr : dr + 1, :], rw[dr : dr + 1, :])
                    for qt in range(2):
                        oi = (b * HPC + hl) * 2 + qt
                        dr = 64 * qt
                        rep = arp.tile([128, TT], F32, tag="rep")
                        nc.tensor.matmul(
                            rep[:], ones128[dr : dr + 1, :], rw_bf[dr : dr + 1, :],
                            start=True, stop=True,
                        )
                        rep_sb = ap.tile([128, TT], F32, tag="repsb")
                        nc.vector.tensor_copy(rep_sb[:], rep[:])
                        tmp = ap.tile([128, TT], F32, tag="tmp")
                        nc.vector.tensor_tensor(tmp[:], yws[qt][:], rep_sb[:], ALU.mult)
                        yfin = ap.tile([128, TT], BF16, tag="yfin")
                        nc.vector.scalar_tensor_tensor(
                            yfin[:], tmp[:], pg, o_store[:, oi, :], ALU.mult, ALU.add
                        )
                        # stage into this batch's a2a bounce: j = tok/128
                        for c4 in range(4):
                            nc.sync.dma_start(
                                a2a_ins[b][qt * 4 + c4, hl * HS : (hl + 1) * HS, :],
                                yfin[:, c4 * 128 : (c4 + 1) * 128],
                            )
                        if b == 0:
                            slot = hl * 2 + qt
                            want = ((slot + 1) * len(a2f)) // 8
                            while a2n < want:
                                a2f[a2n]()
                                a2n += 1
                # batch b fully staged: launch its AllToAll (overlaps the
                # next batch's attention / c_proj chunks)
                nc.gpsimd.collective_compute(
                    "AllToAll",
                    ALU.bypass,
                    replica_groups=[list(range(NCORES))],
                    ins=[a2a_ins[b][:].opt()],
                    outs=[a2a_outs[b][:].opt()],
                )
            arps.close()

        mid.close()  # release whc/ostp SBUF for the cproj weight stream

        # =============== Phase P: c_proj on own token rows.
        # Core j owns tokens [128j, 128j+128) of each batch; out rows
        # 0:128 = batch 0, 128:256 = batch 1. The b=0 chunk of each n
        # only waits on the first collective.
        with (
            tc.tile_pool(name="py", bufs=1) as py,
            tc.tile_pool(name="pw", bufs=5) as pwp,
            tc.tile_pool(name="pp", bufs=4, space="PSUM") as pp,
            tc.tile_pool(name="po", bufs=3) as po,
        ):
            yT0 = py.tile([128, KO, 128], BF16)
            nc.sync.dma_start(
                yT0[:],
                a2a0_out[:]
                .rearrange("i r t -> (i r) t")
                .rearrange("(ko p) t -> p ko t", p=128),
            )
            yT1 = py.tile([128, KO, 128], BF16)
            nc.sync.dma_start(
                yT1[:],
                a2a1_out[:]
                .rearrange("i r t -> (i r) t")
                .rearrange("(ko p) t -> p ko t", p=128),
            )
            yTs = [yT0, yT1]
            NP = C // TT

            def p_chunk(n, half, w_n):
                ps = pp.tile([128, TT], F32, tag="o_ps")
                for ko in range(KO):
                    nc.tensor.matmul(
                        ps[:],
                        yTs[half][:, ko, :],
                        w_n[:, ko, :],
                        start=(ko == 0), stop=(ko == KO - 1),
                    )
                o_t = po.tile([128, TT], F32, tag="o_t")
                nc.vector.tensor_copy(o_t[:], ps[:])
                nc.sync.dma_start(
                    out[half * 128 : (half + 1) * 128, n * TT : (n + 1) * TT], o_t[:]
                )

            # half-0 chunks only wait on the first collective; half-1 chunks
            # are delayed 3 n's behind so the PE has batch-0 work queued
            # while the second collective completes (pw bufs=5 keeps the
            # needed cproj column blocks resident).
            DELAY = 3
            w_tiles = {}
            for n in range(NP):
                w_n = pwp.tile([128, KO, TT], BF16, tag="w_n")
                w_tiles[n] = w_n
                nc.sync.dma_start(
                    w_n[:],
                    cproj[:, n * TT : (n + 1) * TT].rearrange("(ko p) t -> p ko t", p=128),
                )
                p_chunk(n, 0, w_n)
                if n >= DELAY:
                    p_chunk(n - DELAY, 1, w_tiles.pop(n - DELAY))
            for n in range(NP - DELAY, NP):
                p_chunk(n, 1, w_tiles.pop(n))

    _split_multi_waits(nc)
    return nc


def prepare_inputs(inputs):
    """Host-side slicing / casting / transposition. Returns in_maps (one
    dict per core)."""
    f32 = np.float32
    x = np.asarray(inputs["x"], f32)
    audio = np.asarray(inputs["audio_features"], f32)
    rope_cos = np.asarray(inputs["rope_cos"], f32)
    rope_sin = np.asarray(inputs["rope_sin"], f32)
    pad_k = np.asarray(inputs["pad_base_k"], f32)
    pad_v = np.asarray(inputs["pad_base_v"], f32)
    c_attn = np.asarray(inputs["c_attn_w"], f32)
    c_proj = np.asarray(inputs["c_proj_w"], f32)
    adapter_wte = np.asarray(inputs["adapter_wte"], f32)
    rms_gate = np.asarray(inputs["rms_gate_w"], f32)
    rms_key = np.asarray(inputs["rms_key_w"], f32)
    rms_val = np.asarray(inputs["rms_value_w"], f32)
    p_down = np.asarray(inputs["proj_down"], f32)
    p_up = np.asarray(inputs["proj_up"], f32)
    wh_k = np.asarray(inputs["whisper_key_w"], f32)
    wh_v = np.asarray(inputs["whisper_value_w"], f32)
    wh_vb = np.asarray(inputs["whisper_value_b"], f32)

    assert np.array_equal(
        np.asarray(inputs["proj_q128"], f32), np.eye(HS, dtype=f32)
    ) and np.array_equal(
        np.asarray(inputs["proj_q32"], f32), np.eye(NH, dtype=f32)
    ), "general q-reprojection path not implemented"
    mask = np.asarray(inputs["mask"])
    assert mask.shape == (1, 1, T, T)
    assert np.array_equal(
        mask[0, 0], np.tril(np.ones((T, T), dtype=bool))
    ), "only causal mask supported"

    xT = np.ascontiguousarray(x.reshape(BT, C).T).astype(NBF)

    # adapter k/v on host (tiny)
    ms = np.mean(adapter_wte * adapter_wte, axis=-1, keepdims=True)
    prefix = adapter_wte / np.sqrt(ms + EPS) * rms_gate
    aqkv = prefix @ c_attn
    ak = aqkv[:, C : 2 * C].reshape(A_LEN, NH, HS)
    av = aqkv[:, 2 * C :].reshape(A_LEN, NH, HS)

    cosT = np.ascontiguousarray(rope_cos.T)
    sinT = np.ascontiguousarray(rope_sin.T)

    # shared [128,128] triangular mask for the diag-boundary column block
    kk = np.arange(128)[:, None]
    jj = np.arange(128)[None, :]
    masks = np.where(jj >= kk, 0.0, NEG).astype(f32)

    vb_t = np.ascontiguousarray(wh_vb.reshape(NOT, 128).T)
    rmsk_t = np.ascontiguousarray(rms_key.reshape(NOT, 128).T)
    rmsv_t = np.ascontiguousarray(rms_val.reshape(NOT, 128).T)
    padkT_perm = np.ascontiguousarray(pad_k.transpose(0, 2, 1)[:, PERM, :])
    cproj_b = c_proj.astype(NBF)
    aT_full = np.ascontiguousarray(audio.reshape(B * AT, AD).T)  # [1280, 3000]
    # pupk col (u, i) = proj_up[:, 64u + PERM64[i]]
    pupk_all = np.empty((DD, 20 * WHD), f32)
    for u in range(20):
        pupk_all[:, u * WHD : (u + 1) * WHD] = p_up[:, u * WHD + PERM64]

    # pad_v key-tiled [B, 128, 12, 64]: cols 64:128 (non-whisper dims) and
    # cols 0:64 (whisper dims, used as additive base on non-whisper cores)
    pv_pad = np.zeros((B, AT2, HS), f32)
    pv_pad[:, :AT, :] = pad_v
    pv_tiles = pv_pad.reshape(B, NKT, 128, HS).transpose(0, 2, 1, 3)
    padvT_hi = np.ascontiguousarray(pv_tiles[:, :, :, WHD:]).astype(NBF)
    padvT_lo = np.ascontiguousarray(pv_tiles[:, :, :, :WHD]).astype(NBF)
    padvT_lo_zero = np.zeros_like(padvT_lo)

    in_maps = []
    for c in range(NCORES):
        heads = range(HPC * c, HPC * c + HPC)
        wq_c = np.empty((C, HPC * HS), f32)
        wk_c = np.empty((C, HPC * HS), f32)
        wv_c = np.empty((C, HPC * HS), f32)
        akT_c = np.empty((HPC, HS, A_LEN), f32)
        av_c = np.empty((HPC, A_LEN, HS), f32)
        for hl, h in enumerate(heads):
            wq_c[:, hl * HS : (hl + 1) * HS] = c_attn[:, h * HS + PERM]
            wk_c[:, hl * HS : (hl + 1) * HS] = c_attn[:, C + h * HS + PERM]
            wv_c[:, hl * HS : (hl + 1) * HS] = (
                c_attn[:, 2 * C + h * HS : 2 * C + (h + 1) * HS]
            )
            akT_c[hl] = ak[:, h, PERM].T
            av_c[hl] = av[:, h, :]

        wk_core = c * HPC + HPC - 1 < NWH  # all 4 heads whisper-backed
        padkT_c = np.zeros((B, HS, AT2), f32)
        padkT_c[:, :, :AT] = padkT_perm
        if wk_core:
            aT_c = np.empty((AD, B * 300), f32)
            for b in range(B):
                aT_c[:, b * 300 : (b + 1) * 300] = aT_full[
                    :, b * AT + 300 * c : b * AT + 300 * c + 300
                ]
            pupk_c, pupv_c = pupk_all, p_up
            padkT_c[:, 0:32, :] = 0.0
            padkT_c[:, 64:96, :] = 0.0
            padv0_c = padvT_lo_zero
        else:
            aT_c = np.zeros((AD, B * 300), f32)
            pupk_c = np.zeros((DD, 20 * WHD), f32)
            pupv_c = np.zeros((DD, AD), f32)
            padv0_c = padvT_lo

        in_maps.append(
            dict(
                xT=xT,
                wq=wq_c.astype(NBF), wk=wk_c.astype(NBF), wv=wv_c.astype(NBF),
                cosT=cosT, sinT=sinT, masks=masks,
                akT=akT_c.astype(NBF), avd=av_c.astype(NBF),
                aT=aT_c.astype(NBF),
                wkey=wh_k.astype(NBF), wval=wh_v.astype(NBF),
                vbias=vb_t, rmsk=rmsk_t, rmsv=rmsv_t,
                pdown=p_down.astype(NBF),
                pupk=pupk_c.astype(NBF), pupv=pupv_c.astype(NBF),
                padkT=padkT_c.astype(NBF),
                padvT=padvT_hi, padv0=padv0_c,
                cproj=cproj_b,
            )
        )
    return in_maps


def get_program(inputs):
    gf = float(np.asarray(inputs["gating_factor"], np.float32))
    pg = float(np.asarray(inputs["proj_gating"], np.float32))
    key = (gf, pg)
    if key not in _PROG_CACHE:
        _PROG_CACHE[key] = build_program(gf, pg)
    return _PROG_CACHE[key]


def kernel(**inputs) -> np.ndarray:
    nc = get_program(inputs)
    in_maps = prepare_inputs(inputs)
    res = run_bass_kernel_spmd(nc, in_maps, core_ids=list(range(NCORES)))
    # core j rows 0:128 = batch-0 tokens [128j, 128j+128); rows 128:256 = batch 1
    full = np.empty((B, T, C), np.float32)
    for c in range(NCORES):
        r = res.results[c]["out"]
        full[0, 128 * c : 128 * (c + 1)] = r[0:128]
        full[1, 128 * c : 128 * (c + 1)] = r[128:256]
    return full
